# revision 5
# baseline (speedup 1.0000x reference)
"""Cross-attention kernel for trn2 (8 NeuronCores, batch-parallel), v2.

Per batch element b (one per core):
    qT = Wq @ x_b + bq            [64, 2048]
    kT = Wk @ y_b + bk            [64, 2048]
    eT[m, n] = exp(qT[:, n] . kT[:, m] - 6)      (shift cancels in softmax)
    vT[m, c] = (Wv @ y_b)[c, m]                  (bias bv folded into residual:
                                                  out = g*(V A^T)/sums + g*bv + x)
    po[c, n] = sum_m vT[m, c] * eT[m, n]
    out = po * (gamma / sums) + (x_b + gamma*bv)

All matmuls run in bf16 (fp32r measured ~2 cycles/row on HW; bf16 is 1).
x stays exact fp32 for the residual path (gamma=0 gives out == x exactly).

Pipelining: x/y stream in 512-column blocks on two HW DMA queues (sync
engine: weights+y+stores; scalar engine: biases+x) with projections
consuming blocks as they land.  The attention phase is software-pipelined
per 128-key tile so the exp (scalar engine) hides under the PE stream.
"""

import numpy as np

import concourse.bass as bass
import concourse.mybir as mybir
import concourse.tile as tile
from concourse.bass_utils import run_bass_kernel_spmd

F32 = mybir.dt.float32
BF16 = mybir.dt.bfloat16
AF = mybir.ActivationFunctionType
OP = mybir.AluOpType

B, C, N, D = 8, 512, 2048, 64
KC = C // 128     # 4 contraction chunks of 128 over channels
CT = C // 128     # 4 output row tiles of 128 over channels
MT = N // 128     # 16 key tiles of 128
NB = 512          # n-block (query block / column block size)
JB = N // NB      # 4 column blocks
SHIFT = 6.0       # exp(e - SHIFT): keeps sums small; cancels exactly

LAST_EXEC_TIME_NS = None
_CACHE = {}


def _legalize_waits(nc, cap=1):
    """walrus in this toolchain rejects >1 sync wait per instruction;
    hoist excess waits into standalone EventSemaphore instructions on the
    same (in-order) engine queue."""
    n = 0
    for f in nc.m.functions:
        for bb in f.blocks:
            insts = list(bb.instructions)
            out = []
            changed = False
            for ins in insts:
                si = getattr(ins, "sync_info", None)
                waits = list(si.on_wait) if si is not None and si.on_wait else []
                if len(waits) > cap:
                    hoist = waits[: len(waits) - cap]
                    keep = waits[len(waits) - cap:]
                    for w in hoist:
                        es = mybir.InstEventSemaphore(
                            name=nc.get_next_instruction_name()
                        )
                        es.engine = ins.engine
                        es.sync_info = mybir.SyncInfo(on_wait=[w], on_update=[])
                        nc.register_instruction(es, overwrite=True)
                        out.append(es)
                        n += 1
                    si.on_wait = keep
                    changed = True
                out.append(ins)
            if changed:
                bb.instructions = out
    return n


def _bcast_ap(ap, parts):
    """Broadcast a 1-D AP across `parts` partitions (step-0 leading dim)."""
    return bass.AP(tensor=ap.tensor, offset=ap.offset, ap=[[0, parts]] + list(ap.ap))


def _build():
    nc = bass.Bass()

    x_d = nc.dram_tensor("x", [C, N], F32, kind="ExternalInput")
    y_d = nc.dram_tensor("y", [C, N], F32, kind="ExternalInput")
    wqt_d = nc.dram_tensor("wqt", [C, D], F32, kind="ExternalInput")
    bq_d = nc.dram_tensor("bq", [D], F32, kind="ExternalInput")
    wkt_d = nc.dram_tensor("wkt", [C, D], F32, kind="ExternalInput")
    bk_d = nc.dram_tensor("bk", [D], F32, kind="ExternalInput")
    wvt_d = nc.dram_tensor("wvt", [C, C], F32, kind="ExternalInput")
    bv_d = nc.dram_tensor("bv", [C], F32, kind="ExternalInput")
    gamma_d = nc.dram_tensor("gamma", [1], F32, kind="ExternalInput")
    out_d = nc.dram_tensor("out", [C, N], F32, kind="ExternalOutput")

    with tile.TileContext(nc) as tc:
        with (
            nc.allow_low_precision(reason="bf16 matmuls are intentional"),
            tc.tile_pool(name="const", bufs=1) as const,
            tc.tile_pool(name="stg", bufs=2) as stg,
            tc.tile_pool(name="et", bufs=4) as etp,
            tc.tile_pool(name="work", bufs=2) as work,
            tc.tile_pool(name="osb", bufs=4) as osbp,
            tc.tile_pool(name="mm_ps", bufs=3, space="PSUM") as mm_ps,
            tc.tile_pool(name="out_ps", bufs=4, space="PSUM") as out_ps,
            tc.tile_pool(name="sum_ps", bufs=1, space="PSUM") as sum_ps,
        ):
            # ---- scalar-queue DMAs: small consts, then x blocks ----
            bq = const.tile([D, 1], F32)
            nc.scalar.dma_start(out=bq, in_=bq_d.ap().rearrange("d -> d ()"))
            bk = const.tile([D, 1], F32)
            nc.scalar.dma_start(out=bk, in_=bk_d.ap().rearrange("d -> d ()"))
            bv_pk = const.tile([128, KC], F32)
            nc.scalar.dma_start(out=bv_pk, in_=bv_d.ap().rearrange("(k p) -> p k", p=128))
            gam_col = const.tile([128, 1], F32)
            nc.scalar.dma_start(out=gam_col, in_=_bcast_ap(gamma_d.ap(), 128))

            x_sb = []
            for j in range(JB):
                js = slice(j * NB, (j + 1) * NB)
                xj = const.tile([128, KC, NB], F32, name=f"x_sb{j}")
                nc.scalar.dma_start(
                    out=xj, in_=x_d.ap()[:, js].rearrange("(k p) n -> p k n", p=128)
                )
                x_sb.append(xj)

            # ---- sync-queue DMAs: weights + y blocks ----
            wq_stg = stg.tile([128, KC, D], F32, tag="wstg")
            nc.sync.dma_start(out=wq_stg, in_=wqt_d.ap().rearrange("(k p) d -> p k d", p=128))
            wk_stg = stg.tile([128, KC, D], F32, tag="wstg")
            nc.sync.dma_start(out=wk_stg, in_=wkt_d.ap().rearrange("(k p) d -> p k d", p=128))
            y_stg = []
            for j in range(JB):
                js = slice(j * NB, (j + 1) * NB)
                yj = stg.tile([128, KC, NB], F32, tag="ystg", bufs=2)
                nc.sync.dma_start(
                    out=yj, in_=y_d.ap()[:, js].rearrange("(k p) n -> p k n", p=128)
                )
                y_stg.append(yj)
                if j == 0:
                    wv_stg = const.tile([128, KC, C], F32)
                    nc.sync.dma_start(
                        out=wv_stg, in_=wvt_d.ap().rearrange("(k p) c -> p k c", p=128)
                    )

            # ---- weight casts to bf16 (vector) ----
            wq_bf = const.tile([128, KC, D], BF16)
            nc.vector.tensor_copy(out=wq_bf, in_=wq_stg)
            wk_bf = const.tile([128, KC, D], BF16)
            nc.vector.tensor_copy(out=wk_bf, in_=wk_stg)
            wv_bf = const.tile([128, KC, C], BF16)
            nc.vector.tensor_copy(out=wv_bf, in_=wv_stg)

            # gbv = gamma * bv, laid out [128, KC] to match channel tiles
            gbv = const.tile([128, KC], F32)
            nc.vector.tensor_scalar_mul(gbv, bv_pk, gam_col)

            ones_bf = const.tile([128, 128], BF16)
            nc.vector.memset(ones_bf, 1.0)
            neg_shift = const.tile([128, 1], F32)
            nc.vector.memset(neg_shift, -SHIFT)

            qT = [const.tile([D, NB], BF16, name=f"qT{j}") for j in range(JB)]
            kT = [const.tile([D, NB], BF16, name=f"kT{j}") for j in range(JB)]
            vT = [const.tile([128, C], BF16, name=f"vT{m}") for m in range(MT)]
            x_bf = [const.tile([128, KC, NB], BF16, name=f"x_bf{j}") for j in range(JB)]
            y_bf = [const.tile([128, KC, NB], BF16, name=f"y_bf{j}") for j in range(JB)]

            # ---- phase 1+2: per column block, projections as data lands ----
            for j in range(JB):
                # y path: kT block, then 4 vT tiles
                nc.gpsimd.tensor_copy(out=y_bf[j], in_=y_stg[j])
                pk = mm_ps.tile([D, NB], F32, tag="mm")
                for kc in range(KC):
                    nc.tensor.matmul(
                        pk, wk_bf[:, kc, :], y_bf[j][:, kc, :],
                        start=(kc == 0), stop=(kc == KC - 1),
                    )
                nc.scalar.activation(out=kT[j], in_=pk, func=AF.Identity, bias=bk)
                for ml in range(4):
                    mt = j * 4 + ml
                    ms = slice(ml * 128, (ml + 1) * 128)
                    pv = mm_ps.tile([128, C], F32, tag="mm")
                    for kc in range(KC):
                        nc.tensor.matmul(
                            pv, y_bf[j][:, kc, ms], wv_bf[:, kc, :],
                            start=(kc == 0), stop=(kc == KC - 1),
                        )
                    nc.scalar.copy(out=vT[mt], in_=pv)
                # x path: qT block; then fold gamma*bv into the residual copy
                nc.vector.tensor_copy(out=x_bf[j], in_=x_sb[j])
                pq = mm_ps.tile([D, NB], F32, tag="mm")
                for kc in range(KC):
                    nc.tensor.matmul(
                        pq, wq_bf[:, kc, :], x_bf[j][:, kc, :],
                        start=(kc == 0), stop=(kc == KC - 1),
                    )
                nc.scalar.activation(out=qT[j], in_=pq, func=AF.Identity, bias=bq)
                for kc in range(KC):
                    nc.vector.tensor_scalar_add(
                        x_sb[j][:, kc, :], x_sb[j][:, kc, :], gbv[:, kc:kc + 1]
                    )

            # ---- phase 3: attention, software-pipelined per key tile ----
            for nb in range(JB):
                ns = slice(nb * NB, (nb + 1) * NB)
                po = [out_ps.tile([128, NB], F32, tag="out", name=f"po{ct}")
                      for ct in range(CT)]
                spsum = sum_ps.tile([128, NB], F32, tag="sum")
                ets = [None] * MT
                for step in range(MT + 1):
                    if step < MT:
                        ml, j = step % 4, step // 4
                        pe_ = mm_ps.tile([128, NB], F32, tag="mm")
                        nc.tensor.matmul(
                            pe_, kT[j][:, ml * 128:(ml + 1) * 128], qT[nb],
                            start=True, stop=True,
                        )
                        et = etp.tile([128, NB], BF16, tag="et")
                        nc.scalar.activation(out=et, in_=pe_, func=AF.Exp, bias=neg_shift)
                        ets[step] = et
                    if step >= 1:
                        p = step - 1
                        nc.tensor.matmul(
                            spsum, ones_bf, ets[p],
                            start=(p == 0), stop=(p == MT - 1),
                        )
                        for ct in range(CT):
                            cs = slice(ct * 128, (ct + 1) * 128)
                            nc.tensor.matmul(
                                po[ct], vT[p][:, cs], ets[p],
                                start=(p == 0), stop=(p == MT - 1),
                            )
                # normalize + residual + store
                rb = work.tile([128, NB], F32, tag="rb")
                nc.vector.reciprocal(out=rb, in_=spsum)
                nc.vector.tensor_scalar_mul(rb, rb, gam_col)
                for ct in range(CT):
                    cs = slice(ct * 128, (ct + 1) * 128)
                    osb = osbp.tile([128, NB], F32, tag="osb")
                    nc.vector.tensor_tensor(osb, po[ct], rb, OP.mult)
                    nc.vector.tensor_tensor(osb, osb, x_sb[nb][:, ct, :], OP.add)
                    nc.sync.dma_start(out=out_d.ap()[cs, ns], in_=osb)

    _legalize_waits(nc)
    return nc


def kernel(x, y, Wq, bq, Wk, bk, Wv, bv, gamma):
    nc = _CACHE.get("nc")
    if nc is None:
        nc = _build()
        _CACHE["nc"] = nc

    wqt = np.ascontiguousarray(np.asarray(Wq, dtype=np.float32).T)
    wkt = np.ascontiguousarray(np.asarray(Wk, dtype=np.float32).T)
    wvt = np.ascontiguousarray(np.asarray(Wv, dtype=np.float32).T)
    x = np.asarray(x, dtype=np.float32)
    y = np.asarray(y, dtype=np.float32)
    in_maps = []
    for b in range(B):
        in_maps.append({
            "x": np.ascontiguousarray(x[b]),
            "y": np.ascontiguousarray(y[b]),
            "wqt": wqt,
            "bq": np.asarray(bq, dtype=np.float32),
            "wkt": wkt,
            "bk": np.asarray(bk, dtype=np.float32),
            "wvt": wvt,
            "bv": np.asarray(bv, dtype=np.float32),
            "gamma": np.asarray(gamma, dtype=np.float32),
        })

    r = run_bass_kernel_spmd(nc, in_maps, core_ids=list(range(B)))
    global LAST_EXEC_TIME_NS
    LAST_EXEC_TIME_NS = r.exec_time_ns
    return np.stack([r.results[b]["out"] for b in range(B)]).astype(np.float32)
